# revision 32
# baseline (speedup 1.0000x reference)
"""FFF (fast feedforward / soft MoE tree) layer for Trainium2, 8 NeuronCores.

Strategy: data-parallel over the 4096-token batch (512 tokens/core), all
weights replicated. Per core, activations live feature-major in SBUF
([feature partitions, token free-dim]) so every matmul uses native weight
slices as lhsT and 512-token tiles as rhs:

  node phase:  hn^T = relu(W1n^T x^T + b1)       8 x 6 matmuls, N=512, with
               z    = W2bd^T hn^T                the 8 block-diag z matmuls
                                                 interleaved one chain behind
                                                 (z accumulates in out-bank 5,
                                                 idle until the leaf phase)
               c = sigmoid(z + b2); 1-c = sigmoid(-z - b2)   (one ACT table)
               w^T  = exp(Mpath^T ln([c; 1-c] + eps))  ACT chain + one fp32r
                                                       path-matrix matmul
  leaf phase:  per leaf l: hl = relu(W1_l^T x^T + b1_l)   6 matmuls -> PSUM
               hls = hl * w_l (per-token scale via broadcast DMA of w rows)
               out^T += W2_l^T @ hls             6 accumulating matmuls
               (+ leaf_b2 folded in as a rank-64 matmul over w^T -- elided
               when leaf_b2 is all-zero, as it is for this problem)

Schedule notes (all measured on HW):
- The DMA queues stripe each transfer across all 16 HBM engines and share
  ~400 GB/s/core; the x + node-weight stream stays serialized on sync in
  consumption order with COARSE whole-tensor transfers (per-chunk slicing
  shrinks per-partition runs to 1-1.5KB and drops throughput ~30%), while
  small constants issue from scalar mid-node-phase and gpsimd carries only
  what the first chains need. Five dummy matmuls bridge the PE clock-ramp
  window (0.65 -> 2.4 GHz after ~3.4us busy) until x lands; PE gaps >2us
  drop the clock back and cost ~3us more, so the stream must never outrun
  the weight DMAs.
- The gating chain's serial latency (sigmoid/ln/exp with inline 1.28us
  table loads, the path matmul, and the wt DRAM round trip + broadcast,
  ~14us) is hidden by prefilling 14 leaves' first-layer chains; their relus
  run on the DVE so the Scalar queue flows straight through the chain, and
  the wt/wrep DMAs issue from the scalar queue right behind exp (the sync
  queue's leaf-weight stream never blocks behind the round trip).
- out^T [768, 512] accumulates in 6 PSUM banks across all 64 leaves; the
  final leaves drain bank-major and each bank ships in its own 256KB DMA
  alternating sync/scalar (gpsimd's software DGE adds ~2us latency).
Matmul inputs are bf16 (fp32 accumulation in PSUM); the path-matrix matmul
runs fp32r (1 cycle/row vs fp32's 4) and all bias handling stays fp32.
"""

import functools
import os
import sys
from contextlib import ExitStack

import numpy as np
import ml_dtypes

for _p in ("/opt/trn_rl_repo", "/root/.axon_site/_ro/trn_rl_repo"):
    if os.path.isdir(_p) and _p not in sys.path:
        sys.path.insert(0, _p)

import concourse.bass as bass
import concourse.tile as tile
from concourse import bacc, mybir
from concourse.bass_utils import run_bass_kernel_spmd

BF16 = ml_dtypes.bfloat16

DEPTH = 6
IN_DIM = 768
NODE_HIDDEN = 16
LEAF_HIDDEN = 128
OUT_DIM = 768
BATCH = 4096
N_NODES = 63
N_LEAVES = 64
N_CORES = 8
BC = BATCH // N_CORES          # 512 tokens per core
KC = IN_DIM // 128             # 6 contraction chunks
HN = N_NODES * NODE_HIDDEN     # 1008 node-hidden total
NJ = (HN + 127) // 128         # 8 node-hidden partition tiles (last = 112)
OC = OUT_DIM // 128            # 6 output-feature chunks
GL = 8                         # leaves per weight-DMA group (fewer DMA issues)
WG = 4                         # leaves per w-broadcast group
PREFILL = 14                   # leaves prefilled before the first out-chain

# Exposed for test harnesses.
LAST_RESULT = None


def _path_matrix() -> np.ndarray:
    """Mpath [128, 64]: logw = Mpath^T @ [log(c) ; pad ; log(1-c) ; pad].

    c = sigmoid(z). Row n (0..62) selects log(c_n) for leaves in the LEFT
    subtree of node n; row 64+n selects log(1-c_n) for leaves in its RIGHT
    subtree (offset 64, not 63: engine APs must start on a partition
    quadrant). Rows 63 and 127 are zero.
    """
    m = np.zeros((128, N_LEAVES), np.float32)
    for leaf in range(N_LEAVES):
        for lvl in range(DEPTH):
            node = (1 << lvl) - 1 + (leaf >> (DEPTH - lvl))
            right = (leaf >> (DEPTH - 1 - lvl)) & 1
            m[node + (64 if right else 0), leaf] = 1.0
    return m


@functools.lru_cache(maxsize=2)
def _build_nc(with_b2l: bool = False) -> bass.Bass:
    nc = bacc.Bacc()
    f32 = mybir.dt.float32
    f32r = mybir.dt.float32r
    bf16 = mybir.dt.bfloat16

    xt_d = nc.dram_tensor("xt", [128, KC, BC], bf16, kind="ExternalInput")
    w1n_d = nc.dram_tensor("w1n", [128, NJ, KC * 128], bf16, kind="ExternalInput")
    w2bd_d = nc.dram_tensor("w2bd", [128, NJ, N_NODES], bf16, kind="ExternalInput")
    b1n_d = nc.dram_tensor("b1n", [128, NJ], f32, kind="ExternalInput")
    b2sp_d = nc.dram_tensor("b2sp", [N_NODES, 2], f32, kind="ExternalInput")
    mneg_d = nc.dram_tensor("mneg", [128, N_LEAVES], f32r, kind="ExternalInput")
    lw1_d = nc.dram_tensor(
        "lw1", [N_LEAVES // GL, 128, GL * KC * 128], bf16, kind="ExternalInput"
    )
    b1l_d = nc.dram_tensor("b1l", [128, N_LEAVES], f32, kind="ExternalInput")
    lw2_d = nc.dram_tensor(
        "lw2", [N_LEAVES // GL, 128, GL * OUT_DIM], bf16, kind="ExternalInput"
    )
    b2l_d = nc.dram_tensor("b2l", [N_LEAVES, OUT_DIM], bf16, kind="ExternalInput")
    out_d = nc.dram_tensor("outT", [OUT_DIM, BC], f32, kind="ExternalOutput")
    # Staging buffer so the per-token leaf weights can be broadcast-read
    # (partition-step-0 APs need a DRAM source).
    wt_dram = nc.dram_tensor("wt_scratch", [N_LEAVES, BC], bf16)

    act = mybir.ActivationFunctionType

    with tile.TileContext(nc) as tc, ExitStack() as ctx:
        consts = ctx.enter_context(tc.tile_pool(name="consts", bufs=1))
        wpool = ctx.enter_context(tc.tile_pool(name="wpool", bufs=3))
        apool = ctx.enter_context(tc.tile_pool(name="apool", bufs=2))
        ppool = ctx.enter_context(tc.tile_pool(name="ppool", bufs=2, space="PSUM"))
        opool = ctx.enter_context(tc.tile_pool(name="opool", bufs=1, space="PSUM"))

        xt = consts.tile([128, KC, BC], bf16)
        w1n = consts.tile([128, NJ, KC, 128], bf16)

        def dma_w1n(eng, jlo, jhi):
            eng.dma_start(
                out=w1n[:, jlo:jhi, :, :],
                in_=w1n_d[:, jlo:jhi, :].rearrange("p j (c h) -> p j c h", c=KC),
            )

        # DMA model (measured): the queues stripe each transfer across all
        # 16 HBM engines and SHARE ~390-440 GB/s per core, small-run
        # transfers (x: 1KB/partition, w1n: 1.5KB) only sustain ~300 GB/s,
        # and every dma_start costs ~0.65us of issue time plus ~1.6us of
        # DGE + completion-semaphore latency. The x + node-weight stream
        # therefore stays SERIALIZED on sync in consumption order (splitting
        # it across queues just causes contention), sized so each tile's
        # completion semaphore fires just before the PE chain needs it.
        pre = consts.tile([128, BC], f32)
        # fp32r so the path matmul runs 1 cycle/row (vs fp32's 4); same bits
        # as fp32, the tag only relaxes PE input rounding.
        sp = consts.tile([128, BC], f32r)
        # rows 63/127 of pre stay 1.0 -> ln gives 0 there, and Mpath's zero
        # rows ignore them. memsets gate the PE warmup -> gpsimd, whose
        # preamble retires first.
        nc.gpsimd.memset(pre, 1.0)
        epsb = consts.tile([128, 1], f32)
        nc.gpsimd.memset(epsb, 1e-38)
        # COARSE transfers: slicing per-chunk would shrink the per-partition
        # contiguous runs to 1-1.5KB (~290 GB/s); whole-tensor DMAs keep
        # 3-12KB runs (~430 GB/s) and land just-in-time for the chains.
        dma_w1n(nc.sync, 0, 2)
        nc.sync.dma_start(out=xt, in_=xt_d[:])
        dma_w1n(nc.sync, 2, 5)
        dma_w1n(nc.sync, 5, 8)
        # gpsimd: only what the node phase needs early; the other small
        # constants are issued from the scalar queue mid-node-phase (their
        # tiny-run descriptors would otherwise hog DMA-engine slots during
        # the bandwidth-critical x/w1n stream)
        b1n = consts.tile([128, NJ], f32)
        nc.gpsimd.dma_start(out=b1n, in_=b1n_d[:])
        w2bd = consts.tile([128, NJ, N_NODES], bf16)
        nc.gpsimd.dma_start(out=w2bd, in_=w2bd_d[:])
        b2sp = consts.tile([N_NODES, 2], f32)
        mneg = consts.tile([128, N_LEAVES], f32r)
        b1l = consts.tile([128, N_LEAVES], f32)
        late_consts = [(b2sp, b2sp_d), (mneg, mneg_d), (b1l, b1l_d)]
        if with_b2l:
            b2l = consts.tile([N_LEAVES, OUT_DIM], bf16)
            late_consts.append((b2l, b2l_d))

        hn = consts.tile([128, NJ, BC], bf16)
        wt = consts.tile([N_LEAVES, BC], bf16)

        # PE warmup: the HAM clock gate keeps an idle PE at 0.65 GHz and only
        # releases to 2.4 GHz after ~3.4us of sustained activity. The PE sits
        # idle waiting for the first DMAs anyway, so burn that window with
        # dummy matmuls (measured ~1.05us each at the cold clock) sized to
        # end just as the x tile lands (~11.4us; a global init barrier at
        # ~8us floors how early the warmup itself can begin).
        warm = ppool.tile([128, BC], f32, tag="work", name="warm")
        for _ in range(5):
            nc.tensor.matmul(warm[:1, :], epsb, pre, start=True, stop=True)

        # The 6 output PSUM banks are idle through the node phase; bank 5
        # doubles as the z accumulator so the 2-buffer work pool never holds
        # a long-lived tile (its WAR reuse would otherwise stall the leaf
        # prefill behind the sigmoid+table-load chain). b2l's start=True
        # matmul resets the bank afterwards.
        pouts = [
            opool.tile([128, BC], f32, tag=f"out{o}", name=f"pout{o}")
            for o in range(OC)
        ]
        zp = pouts[5]

        # ---- node phase: hn chains with the z chain interleaved one step
        # behind (z_j's matmul needs relu_j's output, which lands ~0.9us
        # after chain j ends -- one chain of skew hides that latency) ----
        def emit_z(j):
            pj = min(128, HN - j * 128)
            nc.tensor.matmul(
                zp[:N_NODES, :],
                w2bd[:pj, j, :],
                hn[:pj, j, :],
                start=(j == 0),
                stop=(j == NJ - 1),
            )

        for j in range(NJ):
            pj = min(128, HN - j * 128)
            ph = ppool.tile([128, BC], f32, tag="work")
            for c in range(KC):
                nc.tensor.matmul(
                    ph[:pj, :],
                    w1n[:, j, c, :pj],
                    xt[:, c, :],
                    start=(c == 0),
                    stop=(c == KC - 1),
                )
            nc.scalar.activation(
                hn[:pj, j, :], ph[:pj, :], act.Relu, bias=b1n[:pj, j : j + 1]
            )
            if j >= 1:
                emit_z(j - 1)
            if 0 <= j - 2 < len(late_consts):
                lt, ld = late_consts[j - 2]
                nc.scalar.dma_start(out=lt, in_=ld[:])
        emit_z(NJ - 1)
        # ---- leaf-phase pipeline helpers ----
        wreps = {}

        def emit_wrep_dma(grp):
            """Broadcast leaf-weight rows (4 leaves) across all partitions.

            Issued from the scalar queue: the exp producing wt runs on that
            same engine, so the wt write + broadcasts queue behind it with
            no cross-queue head-of-line blocking, and the sync queue's
            leaf-weight stream never stalls behind the gating round trip.
            (gpsimd would be idle but its software-DGE path adds multi-us
            latency; scalar's hardware DGE does not.)
            """
            wrep = wpool.tile([128, WG, BC], bf16, tag="wrep", bufs=6, name="wrep")
            src = bass.AP(
                tensor=wt_dram,
                offset=grp * WG * BC,
                ap=[[0, 128], [BC, WG], [1, BC]],
            )
            nc.scalar.dma_start(out=wrep, in_=src)
            wreps[grp] = wrep

        lwg = {}

        def emit_lw1_dma(g):
            w1t = wpool.tile([128, GL, KC, 128], bf16, tag="lw1", bufs=2, name="w1t")
            nc.sync.dma_start(
                out=w1t,
                in_=lw1_d[g].rearrange("p (i c h) -> p i c h", i=GL, c=KC),
            )
            return w1t

        def emit_lw2_dma(g):
            w2t = wpool.tile([128, GL, OUT_DIM], bf16, tag="lw2", bufs=2, name="w2t")
            nc.sync.dma_start(
                out=w2t, in_=lw2_d[g].rearrange("p (i o) -> p i o", i=GL)
            )
            return w2t

        def front_a_mm(leaf):
            """Weight DMAs (grouped) + hl matmuls for one leaf."""
            if leaf % WG == 3 and leaf >= 15 and leaf // WG + 3 < N_LEAVES // WG:
                # prefetch the broadcast group ~9 leaves ahead of its first
                # consumer (groups 0-5 are emitted explicitly after the wt
                # write; same-queue order carries the RAW dep on wt_dram --
                # triggers from prefill leaves would issue BEFORE the wt
                # write and read stale DRAM)
                emit_wrep_dma(leaf // WG + 3)
            g = leaf // GL
            if leaf % GL == 0:
                lwg[g] = [emit_lw1_dma(g), emit_lw2_dma(g)]
            w1t = lwg[g][0]
            i = leaf % GL

            ph = ppool.tile([128, BC], f32, tag="work", name="ph")
            for c in range(KC):
                nc.tensor.matmul(
                    ph,
                    w1t[:, i, c, :],
                    xt[:, c, :],
                    start=(c == 0),
                    stop=(c == KC - 1),
                )
            return (ph, leaf)

        def front_a_act(st):
            """hl = relu(ph + b1l) on the Scalar engine."""
            ph, leaf = st
            hl = apool.tile([128, BC], bf16, tag="hl", bufs=16, name="hl")
            nc.scalar.activation(hl, ph, act.Relu, bias=b1l[:, leaf : leaf + 1])
            return (hl, leaf)

        def front_a_act_dve(st):
            """hl = relu(ph + b1l) on the Vector engine -- used for the
            prefill leaves so the Scalar queue stays free for the gating
            sigmoid -> ln -> exp chain."""
            ph, leaf = st
            hl = apool.tile([128, BC], bf16, tag="hl", bufs=16, name="hl")
            nc.vector.tensor_scalar(
                hl, ph, b1l[:, leaf : leaf + 1], 0.0,
                mybir.AluOpType.add, mybir.AluOpType.max,
            )
            return (hl, leaf)

        def front_a(leaf):
            return front_a_act(front_a_mm(leaf))

        def front_b(st):
            """Per-token leaf-weight scale (needs wrep for the leaf's group)."""
            hl, leaf = st
            hls = apool.tile([128, BC], bf16, tag="hls", bufs=16, name="hls")
            nc.vector.tensor_mul(hls, hl, wreps[leaf // WG][:, leaf % WG, :])
            return (hls, leaf)

        banks_started = [with_b2l]

        def leaf_out(pend, last=False):
            p_hls, p_leaf = pend
            p_w2t = lwg[p_leaf // GL][1]
            start = not banks_started[0]
            banks_started[0] = True
            for o in range(OC):
                nc.tensor.matmul(
                    pouts[o],
                    p_w2t[:, p_leaf % GL, o * 128 : (o + 1) * 128],
                    p_hls,
                    start=start,
                    stop=last,
                )

        # The gating ACT chain is emitted BEFORE the prefilled leaves so it
        # isn't queued behind their relu ops on the Scalar engine.
        # pre[0:63]  = c = sigmoid(zp + b2)
        # pre[64:127] = sigmoid(-zp - b2) = 1 - c  (same ACT table -- a DVE
        # 1-c would serialize cross-engine behind the sigmoid instead)
        # sp = ln(pre + 1e-38) in ONE activation over all 128 partitions --
        # the +eps bias keeps saturated gates finite (ln(1e-38) = -87.5,
        # whose exp underflows to the correct 0 leaf weight, and never
        # produces inf/NaN in the path matmul).
        nc.scalar.activation(
            pre[0:N_NODES, :], zp[:N_NODES, :], act.Sigmoid, bias=b2sp[:, 1:2]
        )
        nc.scalar.activation(
            pre[64 : 64 + N_NODES, :], zp[:N_NODES, :], act.Sigmoid,
            bias=b2sp[:, 0:1], scale=-1.0,
        )
        nc.scalar.activation(sp, pre, act.Ln, bias=epsb)

        # Prefill leaves: their hl matmuls keep PE busy while the gating
        # chain (sigmoid/ln/exp incl. their inline 1.28us table loads, the
        # path matmul, and the wt DRAM round trip for the broadcast, ~13us
        # serial) produces the leaf weights. The path matmul is emitted five
        # hl chains into the prefill so the in-order PE queue reaches it
        # after ln has retired; the prefill relus run on the DVE so the
        # Scalar queue flows straight through sigmoid -> ln -> exp -> DMAs.
        mms = [front_a_mm(leaf) for leaf in range(5)]

        # fp32r runs the PE at 1 cycle/row (vs fp32's 4) and keeps ~19
        # mantissa bits -- ample for summing six ln-gate terms.
        lwp = ppool.tile([128, BC], f32, tag="work", name="lwp")
        nc.tensor.matmul(lwp[:N_LEAVES, :], mneg, sp, start=True, stop=True)

        mms += [front_a_mm(leaf) for leaf in range(5, PREFILL)]
        nc.scalar.activation(wt, lwp[:N_LEAVES, :], act.Exp)
        nc.scalar.dma_start(out=wt_dram[:], in_=wt)
        prefill = [front_a_act_dve(st) for st in mms]
        for grp in range(6):
            emit_wrep_dma(grp)
        pending = [front_b(st) for st in prefill]

        if with_b2l:
            # leaf_b2 contribution: out^T += b2l^T @ w^T (starts the banks)
            for o in range(OC):
                nc.tensor.matmul(
                    pouts[o], b2l[:, o * 128 : (o + 1) * 128], wt,
                    start=True, stop=False,
                )

        # steady state: PREFILL-leaf software-pipeline skew
        for leaf in range(PREFILL, N_LEAVES):
            pending.append(front_b(front_a(leaf)))
            leaf_out(pending.pop(0))

        # Final leaves drain BANK-major: each output bank finishes all its
        # remaining accumulations consecutively, then its PSUM->SBUF copy
        # starts while later banks are still accumulating. Each bank ships
        # in its own 256KB DMA on a rotating queue so the LAST bank -- the
        # critical path after the final matmul -- rides an idle queue.
        # sync/scalar only: gpsimd's software-DGE adds ~2us latency per DMA
        # (DMA cannot read PSUM directly, hence the per-bank staging copy)
        osb = apool.tile([128, OC, BC], f32, tag="osb", bufs=1, name="osb")
        out_queues = [nc.sync, nc.scalar, nc.sync, nc.scalar, nc.sync, nc.scalar]
        for o in range(OC):
            for idx, (p_hls, p_leaf) in enumerate(pending):
                nc.tensor.matmul(
                    pouts[o],
                    lwg[p_leaf // GL][1][:, p_leaf % GL, o * 128 : (o + 1) * 128],
                    p_hls,
                    start=False,
                    stop=(idx == len(pending) - 1),
                )
            nc.vector.tensor_copy(osb[:, o, :], pouts[o])
            out_queues[o].dma_start(
                out=out_d[o * 128 : (o + 1) * 128, :],
                in_=osb[:, o, :],
            )

    nc.compile()
    return nc


def _to_bf16(a: np.ndarray) -> np.ndarray:
    return np.ascontiguousarray(a, dtype=np.float32).astype(BF16)


def prep_inputs(x, node_w1, node_b1, node_w2, node_b2,
                leaf_w1, leaf_b1, leaf_w2, leaf_b2):
    """Host-side layout prep. Returns (shared weight map, per-core x maps)."""
    x = np.asarray(x, np.float32)
    node_w1 = np.asarray(node_w1, np.float32)
    node_b1 = np.asarray(node_b1, np.float32)
    node_w2 = np.asarray(node_w2, np.float32)
    node_b2 = np.asarray(node_b2, np.float32)
    leaf_w1 = np.asarray(leaf_w1, np.float32)
    leaf_b1 = np.asarray(leaf_b1, np.float32)
    leaf_w2 = np.asarray(leaf_w2, np.float32)
    leaf_b2 = np.asarray(leaf_b2, np.float32)

    # node W1 -> [128, NJ, KC*128]: (p, j, c*128+h') = W1n[c*128+p, j*128+h']
    # (W1n [768, 1008] zero-padded to 1024 columns)
    w1n_flat = node_w1.transpose(1, 0, 2).reshape(IN_DIM, HN)
    w1n_pad = np.zeros((IN_DIM, NJ * 128), np.float32)
    w1n_pad[:, :HN] = w1n_flat
    w1n = w1n_pad.reshape(KC, 128, NJ, 128).transpose(1, 2, 0, 3)
    w1n = w1n.reshape(128, NJ, KC * 128)
    # block-diagonal node W2 [HN, 63], padded to 1024 rows -> [128, NJ, 63]
    w2bd = np.zeros((NJ * 128, N_NODES), np.float32)
    for n in range(N_NODES):
        w2bd[n * NODE_HIDDEN : (n + 1) * NODE_HIDDEN, n] = node_w2[n, :, 0]
    w2bd = w2bd.reshape(NJ, 128, N_NODES).transpose(1, 0, 2)
    # node b1 -> [128, NJ]
    b1n = np.zeros((NJ * 128,), np.float32)
    b1n[:HN] = node_b1.reshape(-1)
    b1n = b1n.reshape(NJ, 128).T
    b2 = node_b2[:, 0]
    b2sp = np.stack([-b2, b2], axis=1)  # [63, 2]

    # leaf W1 grouped GL leaves per DMA: [NG, 128, GL*KC*128] with
    # (g, p, (i, c, h)) = leaf_w1[g*GL+i, c*128+p, h]
    ng = N_LEAVES // GL
    lw1 = leaf_w1.reshape(ng, GL, KC, 128, LEAF_HIDDEN).transpose(0, 3, 1, 2, 4)
    lw1 = lw1.reshape(ng, 128, GL * KC * 128)
    # leaf W2 grouped: [NG, 128, GL*OUT] with (g, p, (i, o)) = leaf_w2[g*GL+i, p, o]
    lw2 = leaf_w2.reshape(ng, GL, LEAF_HIDDEN, OUT_DIM).transpose(0, 2, 1, 3)
    lw2 = lw2.reshape(ng, 128, GL * OUT_DIM)
    b1l = leaf_b1.T  # [128, 64]

    shared = {
        "w1n": _to_bf16(w1n),
        "w2bd": _to_bf16(w2bd),
        "b1n": np.ascontiguousarray(b1n, np.float32),
        "b2sp": np.ascontiguousarray(b2sp, np.float32),
        "mneg": _path_matrix(),
        "lw1": _to_bf16(lw1),
        "b1l": np.ascontiguousarray(b1l, np.float32),
        "lw2": _to_bf16(lw2),
        "b2l": _to_bf16(leaf_b2),
    }
    xts = []
    for c in range(N_CORES):
        xc = x[c * BC : (c + 1) * BC].T  # [768, 512]
        xt = xc.reshape(KC, 128, BC).transpose(1, 0, 2)
        xts.append(_to_bf16(xt))
    return shared, xts


def kernel(**inputs) -> np.ndarray:
    global LAST_RESULT
    shared, xts = prep_inputs(**inputs)
    # leaf_b2 is all-zero for this problem's inputs; the build skips its six
    # rank-64 matmuls then (the first leaf out-chain starts the PSUM banks).
    nc = _build_nc(bool(np.any(np.asarray(inputs["leaf_b2"], np.float32))))
    in_maps = [{**shared, "xt": xts[c]} for c in range(N_CORES)]
    trace = os.environ.get("FFF_TRACE", "0") == "1"
    res = run_bass_kernel_spmd(nc, in_maps, list(range(N_CORES)), trace=trace)
    LAST_RESULT = res
    out = np.empty((BATCH, OUT_DIM), np.float32)
    for c in range(N_CORES):
        out[c * BC : (c + 1) * BC, :] = res.results[c]["outT"].T
    return out


# revision 33
# speedup vs baseline: 1.1937x; 1.1937x over previous
"""FFF (fast feedforward / soft MoE tree) layer for Trainium2, 8 NeuronCores.

Strategy: data-parallel over the 4096-token batch (512 tokens/core), all
weights replicated. Per core, activations live feature-major in SBUF
([feature partitions, token free-dim]) so every matmul uses native weight
slices as lhsT and 512-token tiles as rhs:

  node phase:  hn^T = relu(W1n^T x^T + b1)       8 x 6 matmuls, N=512, with
               z    = W2bd^T hn^T                the 8 block-diag z matmuls
                                                 interleaved one chain behind
                                                 (z accumulates in out-bank 5,
                                                 idle until the leaf phase)
               c = sigmoid(z + b2); 1-c = sigmoid(-z - b2)   (one ACT table)
               w^T  = exp(Mpath^T ln([c; 1-c] + eps))  ACT chain + one fp32r
                                                       path-matrix matmul
  leaf phase:  per leaf l: hl = relu(W1_l^T x^T + b1_l)   6 matmuls -> PSUM
               hls = hl * w_l (per-token scale via broadcast DMA of w rows)
               out^T += W2_l^T @ hls             6 accumulating matmuls
               (+ leaf_b2 folded in as a rank-64 matmul over w^T -- elided
               when leaf_b2 is all-zero, as it is for this problem)

Schedule notes (all measured on HW):
- The DMA queues stripe each transfer across all 16 HBM engines and share
  ~400 GB/s/core; the x + node-weight stream stays serialized on sync in
  consumption order with COARSE whole-tensor transfers (per-chunk slicing
  shrinks per-partition runs to 1-1.5KB and drops throughput ~30%), while
  small constants issue from scalar mid-node-phase and gpsimd carries only
  what the first chains need. Five dummy matmuls bridge the PE clock-ramp
  window (0.65 -> 2.4 GHz after ~3.4us busy) until x lands; PE gaps >2us
  drop the clock back and cost ~3us more, so the stream must never outrun
  the weight DMAs.
- The gating chain's serial latency (sigmoid/ln/exp with inline 1.28us
  table loads, the path matmul, and the wt DRAM round trip + broadcast,
  ~14us) is hidden by prefilling 14 leaves' first-layer chains; their relus
  run on the DVE so the Scalar queue flows straight through the chain, and
  the wt/wrep DMAs issue from the scalar queue right behind exp (the sync
  queue's leaf-weight stream never blocks behind the round trip).
- out^T [768, 512] accumulates in 6 PSUM banks across all 64 leaves; the
  final leaves drain bank-major and each bank ships in its own 256KB DMA
  alternating sync/scalar (gpsimd's software DGE adds ~2us latency).
Matmul inputs are bf16 (fp32 accumulation in PSUM); the path-matrix matmul
runs fp32r (1 cycle/row vs fp32's 4) and all bias handling stays fp32.
"""

import functools
import os
import sys
from contextlib import ExitStack

import numpy as np
import ml_dtypes

for _p in ("/opt/trn_rl_repo", "/root/.axon_site/_ro/trn_rl_repo"):
    if os.path.isdir(_p) and _p not in sys.path:
        sys.path.insert(0, _p)

import concourse.bass as bass
import concourse.tile as tile
from concourse import bacc, mybir
from concourse.bass_utils import run_bass_kernel_spmd

BF16 = ml_dtypes.bfloat16

DEPTH = 6
IN_DIM = 768
NODE_HIDDEN = 16
LEAF_HIDDEN = 128
OUT_DIM = 768
BATCH = 4096
N_NODES = 63
N_LEAVES = 64
N_CORES = 8
BC = BATCH // N_CORES          # 512 tokens per core
KC = IN_DIM // 128             # 6 contraction chunks
HN = N_NODES * NODE_HIDDEN     # 1008 node-hidden total
NJ = (HN + 127) // 128         # 8 node-hidden partition tiles (last = 112)
OC = OUT_DIM // 128            # 6 output-feature chunks
GL = 8                         # leaves per weight-DMA group (fewer DMA issues)
WG = 4                         # leaves per w-broadcast group
PREFILL = 14                   # leaves prefilled before the first out-chain

# Exposed for test harnesses.
LAST_RESULT = None


def _path_matrix() -> np.ndarray:
    """Mpath [128, 64]: logw = Mpath^T @ [log(c) ; pad ; log(1-c) ; pad].

    c = sigmoid(z). Row n (0..62) selects log(c_n) for leaves in the LEFT
    subtree of node n; row 64+n selects log(1-c_n) for leaves in its RIGHT
    subtree (offset 64, not 63: engine APs must start on a partition
    quadrant). Rows 63 and 127 are zero.
    """
    m = np.zeros((128, N_LEAVES), np.float32)
    for leaf in range(N_LEAVES):
        for lvl in range(DEPTH):
            node = (1 << lvl) - 1 + (leaf >> (DEPTH - lvl))
            right = (leaf >> (DEPTH - 1 - lvl)) & 1
            m[node + (64 if right else 0), leaf] = 1.0
    return m


@functools.lru_cache(maxsize=2)
def _build_nc(with_b2l: bool = False) -> bass.Bass:
    nc = bacc.Bacc()
    f32 = mybir.dt.float32
    f32r = mybir.dt.float32r
    bf16 = mybir.dt.bfloat16

    xt_d = nc.dram_tensor("xt", [128, KC, BC], bf16, kind="ExternalInput")
    w1n_d = nc.dram_tensor("w1n", [128, NJ, KC * 128], bf16, kind="ExternalInput")
    w2bd_d = nc.dram_tensor("w2bd", [128, NJ, N_NODES], bf16, kind="ExternalInput")
    b1n_d = nc.dram_tensor("b1n", [128, NJ], f32, kind="ExternalInput")
    b2sp_d = nc.dram_tensor("b2sp", [N_NODES, 2], f32, kind="ExternalInput")
    mneg_d = nc.dram_tensor("mneg", [128, N_LEAVES], f32r, kind="ExternalInput")
    lw1_d = nc.dram_tensor(
        "lw1", [N_LEAVES // GL, 128, GL * KC * 128], bf16, kind="ExternalInput"
    )
    b1l_d = nc.dram_tensor("b1l", [128, N_LEAVES], f32, kind="ExternalInput")
    lw2_d = nc.dram_tensor(
        "lw2", [N_LEAVES // GL, 128, GL * OUT_DIM], bf16, kind="ExternalInput"
    )
    b2l_d = nc.dram_tensor("b2l", [N_LEAVES, OUT_DIM], bf16, kind="ExternalInput")
    # bf16 output halves the tail DMA bytes; the host casts back to fp32
    # (adds ~2e-3 rel quantization error on top of 4.7e-3, gate is 2e-2)
    out_d = nc.dram_tensor("outT", [OUT_DIM, BC], bf16, kind="ExternalOutput")
    # Staging buffer so the per-token leaf weights can be broadcast-read
    # (partition-step-0 APs need a DRAM source).
    wt_dram = nc.dram_tensor("wt_scratch", [N_LEAVES, BC], bf16)

    act = mybir.ActivationFunctionType

    with tile.TileContext(nc) as tc, ExitStack() as ctx:
        consts = ctx.enter_context(tc.tile_pool(name="consts", bufs=1))
        wpool = ctx.enter_context(tc.tile_pool(name="wpool", bufs=3))
        apool = ctx.enter_context(tc.tile_pool(name="apool", bufs=2))
        ppool = ctx.enter_context(tc.tile_pool(name="ppool", bufs=2, space="PSUM"))
        opool = ctx.enter_context(tc.tile_pool(name="opool", bufs=1, space="PSUM"))

        xt = consts.tile([128, KC, BC], bf16)
        w1n = consts.tile([128, NJ, KC, 128], bf16)

        def dma_w1n(eng, jlo, jhi):
            eng.dma_start(
                out=w1n[:, jlo:jhi, :, :],
                in_=w1n_d[:, jlo:jhi, :].rearrange("p j (c h) -> p j c h", c=KC),
            )

        # DMA model (measured): the queues stripe each transfer across all
        # 16 HBM engines and SHARE ~390-440 GB/s per core, small-run
        # transfers (x: 1KB/partition, w1n: 1.5KB) only sustain ~300 GB/s,
        # and every dma_start costs ~0.65us of issue time plus ~1.6us of
        # DGE + completion-semaphore latency. The x + node-weight stream
        # therefore stays SERIALIZED on sync in consumption order (splitting
        # it across queues just causes contention), sized so each tile's
        # completion semaphore fires just before the PE chain needs it.
        pre = consts.tile([128, BC], f32)
        # fp32r so the path matmul runs 1 cycle/row (vs fp32's 4); same bits
        # as fp32, the tag only relaxes PE input rounding.
        sp = consts.tile([128, BC], f32r)
        # rows 63/127 of pre stay 1.0 -> ln gives 0 there, and Mpath's zero
        # rows ignore them. memsets gate the PE warmup -> gpsimd, whose
        # preamble retires first.
        nc.gpsimd.memset(pre, 1.0)
        epsb = consts.tile([128, 1], f32)
        nc.gpsimd.memset(epsb, 1e-38)
        # COARSE transfers: slicing per-chunk would shrink the per-partition
        # contiguous runs to 1-1.5KB (~290 GB/s); whole-tensor DMAs keep
        # 3-12KB runs (~430 GB/s) and land just-in-time for the chains.
        nc.sync.dma_start(out=xt, in_=xt_d[:])
        dma_w1n(nc.sync, 0, 2)
        dma_w1n(nc.sync, 2, 5)
        dma_w1n(nc.sync, 5, 8)
        # gpsimd: only what the node phase needs early; the other small
        # constants are issued from the scalar queue mid-node-phase (their
        # tiny-run descriptors would otherwise hog DMA-engine slots during
        # the bandwidth-critical x/w1n stream)
        b1n = consts.tile([128, NJ], f32)
        nc.gpsimd.dma_start(out=b1n, in_=b1n_d[:])
        w2bd = consts.tile([128, NJ, N_NODES], bf16)
        nc.gpsimd.dma_start(out=w2bd, in_=w2bd_d[:])
        b2sp = consts.tile([N_NODES, 2], f32)
        mneg = consts.tile([128, N_LEAVES], f32r)
        b1l = consts.tile([128, N_LEAVES], f32)
        late_consts = [(b2sp, b2sp_d), (mneg, mneg_d), (b1l, b1l_d)]
        if with_b2l:
            b2l = consts.tile([N_LEAVES, OUT_DIM], bf16)
            late_consts.append((b2l, b2l_d))

        hn = consts.tile([128, NJ, BC], bf16)
        wt = consts.tile([N_LEAVES, BC], bf16)

        # PE warmup: the HAM clock gate keeps an idle PE at 0.65 GHz and only
        # releases to 2.4 GHz after ~3.4us of sustained activity. The PE sits
        # idle waiting for the first DMAs anyway, so burn that window with
        # dummy matmuls (measured ~1.05us each at the cold clock) sized to
        # end just as the x tile lands (~11.4us; a global init barrier at
        # ~8us floors how early the warmup itself can begin).
        warm = ppool.tile([128, BC], f32, tag="work", name="warm")
        for _ in range(4):
            nc.tensor.matmul(warm[:1, :], epsb, pre, start=True, stop=True)

        # The 6 output PSUM banks are idle through the node phase; bank 5
        # doubles as the z accumulator so the 2-buffer work pool never holds
        # a long-lived tile (its WAR reuse would otherwise stall the leaf
        # prefill behind the sigmoid+table-load chain). b2l's start=True
        # matmul resets the bank afterwards.
        pouts = [
            opool.tile([128, BC], f32, tag=f"out{o}", name=f"pout{o}")
            for o in range(OC)
        ]
        zp = pouts[5]

        # ---- node phase: hn chains with the z chain interleaved one step
        # behind (z_j's matmul needs relu_j's output, which lands ~0.9us
        # after chain j ends -- one chain of skew hides that latency) ----
        def emit_z(j):
            pj = min(128, HN - j * 128)
            nc.tensor.matmul(
                zp[:N_NODES, :],
                w2bd[:pj, j, :],
                hn[:pj, j, :],
                start=(j == 0),
                stop=(j == NJ - 1),
            )

        for j in range(NJ):
            pj = min(128, HN - j * 128)
            ph = ppool.tile([128, BC], f32, tag="work")
            for c in range(KC):
                nc.tensor.matmul(
                    ph[:pj, :],
                    w1n[:, j, c, :pj],
                    xt[:, c, :],
                    start=(c == 0),
                    stop=(c == KC - 1),
                )
            nc.scalar.activation(
                hn[:pj, j, :], ph[:pj, :], act.Relu, bias=b1n[:pj, j : j + 1]
            )
            if j >= 1:
                emit_z(j - 1)
            if 0 <= j - 2 < len(late_consts):
                lt, ld = late_consts[j - 2]
                nc.scalar.dma_start(out=lt, in_=ld[:])
        emit_z(NJ - 1)
        # ---- leaf-phase pipeline helpers ----
        wreps = {}

        def emit_wrep_dma(grp):
            """Broadcast leaf-weight rows (4 leaves) across all partitions.

            Issued from the scalar queue: the exp producing wt runs on that
            same engine, so the wt write + broadcasts queue behind it with
            no cross-queue head-of-line blocking, and the sync queue's
            leaf-weight stream never stalls behind the gating round trip.
            (gpsimd would be idle but its software-DGE path adds multi-us
            latency; scalar's hardware DGE does not.)
            """
            wrep = wpool.tile([128, WG, BC], bf16, tag="wrep", bufs=6, name="wrep")
            src = bass.AP(
                tensor=wt_dram,
                offset=grp * WG * BC,
                ap=[[0, 128], [BC, WG], [1, BC]],
            )
            nc.scalar.dma_start(out=wrep, in_=src)
            wreps[grp] = wrep

        lwg = {}

        def emit_lw1_dma(g):
            w1t = wpool.tile([128, GL, KC, 128], bf16, tag="lw1", bufs=2, name="w1t")
            nc.sync.dma_start(
                out=w1t,
                in_=lw1_d[g].rearrange("p (i c h) -> p i c h", i=GL, c=KC),
            )
            return w1t

        def emit_lw2_dma(g):
            w2t = wpool.tile([128, GL, OUT_DIM], bf16, tag="lw2", bufs=2, name="w2t")
            nc.sync.dma_start(
                out=w2t, in_=lw2_d[g].rearrange("p (i o) -> p i o", i=GL)
            )
            return w2t

        def front_a_mm(leaf):
            """Weight DMAs (grouped) + hl matmuls for one leaf."""
            if leaf % WG == 3 and leaf >= 15 and leaf // WG + 3 < N_LEAVES // WG:
                # prefetch the broadcast group ~9 leaves ahead of its first
                # consumer (groups 0-5 are emitted explicitly after the wt
                # write; same-queue order carries the RAW dep on wt_dram --
                # triggers from prefill leaves would issue BEFORE the wt
                # write and read stale DRAM)
                emit_wrep_dma(leaf // WG + 3)
            g = leaf // GL
            if leaf % GL == 0:
                lwg[g] = [emit_lw1_dma(g), emit_lw2_dma(g)]
            w1t = lwg[g][0]
            i = leaf % GL

            ph = ppool.tile([128, BC], f32, tag="work", name="ph")
            for c in range(KC):
                nc.tensor.matmul(
                    ph,
                    w1t[:, i, c, :],
                    xt[:, c, :],
                    start=(c == 0),
                    stop=(c == KC - 1),
                )
            return (ph, leaf)

        def front_a_act(st):
            """hl = relu(ph + b1l) on the Scalar engine."""
            ph, leaf = st
            hl = apool.tile([128, BC], bf16, tag="hl", bufs=16, name="hl")
            nc.scalar.activation(hl, ph, act.Relu, bias=b1l[:, leaf : leaf + 1])
            return (hl, leaf)

        def front_a_act_dve(st):
            """hl = relu(ph + b1l) on the Vector engine -- used for the
            prefill leaves so the Scalar queue stays free for the gating
            sigmoid -> ln -> exp chain."""
            ph, leaf = st
            hl = apool.tile([128, BC], bf16, tag="hl", bufs=16, name="hl")
            nc.vector.tensor_scalar(
                hl, ph, b1l[:, leaf : leaf + 1], 0.0,
                mybir.AluOpType.add, mybir.AluOpType.max,
            )
            return (hl, leaf)

        def front_a(leaf):
            return front_a_act(front_a_mm(leaf))

        def front_b(st):
            """Per-token leaf-weight scale (needs wrep for the leaf's group)."""
            hl, leaf = st
            hls = apool.tile([128, BC], bf16, tag="hls", bufs=16, name="hls")
            nc.vector.tensor_mul(hls, hl, wreps[leaf // WG][:, leaf % WG, :])
            return (hls, leaf)

        banks_started = [with_b2l]

        def leaf_out(pend, last=False):
            p_hls, p_leaf = pend
            p_w2t = lwg[p_leaf // GL][1]
            start = not banks_started[0]
            banks_started[0] = True
            for o in range(OC):
                nc.tensor.matmul(
                    pouts[o],
                    p_w2t[:, p_leaf % GL, o * 128 : (o + 1) * 128],
                    p_hls,
                    start=start,
                    stop=last,
                )

        # The gating ACT chain is emitted BEFORE the prefilled leaves so it
        # isn't queued behind their relu ops on the Scalar engine.
        # pre[0:63]  = c = sigmoid(zp + b2)
        # pre[64:127] = sigmoid(-zp - b2) = 1 - c  (same ACT table -- a DVE
        # 1-c would serialize cross-engine behind the sigmoid instead)
        # sp = ln(pre + 1e-38) in ONE activation over all 128 partitions --
        # the +eps bias keeps saturated gates finite (ln(1e-38) = -87.5,
        # whose exp underflows to the correct 0 leaf weight, and never
        # produces inf/NaN in the path matmul).
        nc.scalar.activation(
            pre[0:N_NODES, :], zp[:N_NODES, :], act.Sigmoid, bias=b2sp[:, 1:2]
        )
        nc.scalar.activation(
            pre[64 : 64 + N_NODES, :], zp[:N_NODES, :], act.Sigmoid,
            bias=b2sp[:, 0:1], scale=-1.0,
        )
        nc.scalar.activation(sp, pre, act.Ln, bias=epsb)

        # Prefill leaves: their hl matmuls keep PE busy while the gating
        # chain (sigmoid/ln/exp incl. their inline 1.28us table loads, the
        # path matmul, and the wt DRAM round trip for the broadcast, ~13us
        # serial) produces the leaf weights. The path matmul is emitted five
        # hl chains into the prefill so the in-order PE queue reaches it
        # after ln has retired; the prefill relus run on the DVE so the
        # Scalar queue flows straight through sigmoid -> ln -> exp -> DMAs.
        mms = [front_a_mm(leaf) for leaf in range(5)]

        # fp32r runs the PE at 1 cycle/row (vs fp32's 4) and keeps ~19
        # mantissa bits -- ample for summing six ln-gate terms.
        lwp = ppool.tile([128, BC], f32, tag="work", name="lwp")
        nc.tensor.matmul(lwp[:N_LEAVES, :], mneg, sp, start=True, stop=True)

        mms += [front_a_mm(leaf) for leaf in range(5, PREFILL)]
        nc.scalar.activation(wt, lwp[:N_LEAVES, :], act.Exp)
        nc.scalar.dma_start(out=wt_dram[:], in_=wt)
        prefill = [front_a_act_dve(st) for st in mms]
        for grp in range(6):
            emit_wrep_dma(grp)
        pending = [front_b(st) for st in prefill]

        if with_b2l:
            # leaf_b2 contribution: out^T += b2l^T @ w^T (starts the banks)
            for o in range(OC):
                nc.tensor.matmul(
                    pouts[o], b2l[:, o * 128 : (o + 1) * 128], wt,
                    start=True, stop=False,
                )

        # steady state: PREFILL-leaf software-pipeline skew
        for leaf in range(PREFILL, N_LEAVES):
            pending.append(front_b(front_a(leaf)))
            leaf_out(pending.pop(0))

        # Final leaves drain BANK-major: each output bank finishes all its
        # remaining accumulations consecutively, then its PSUM->SBUF copy
        # starts while later banks are still accumulating. Each bank ships
        # in its own 256KB DMA on a rotating queue so the LAST bank -- the
        # critical path after the final matmul -- rides an idle queue.
        # sync/scalar only: gpsimd's software-DGE adds ~2us latency per DMA
        # (DMA cannot read PSUM directly, hence the per-bank staging copy)
        osb = apool.tile([128, OC, BC], bf16, tag="osb", bufs=1, name="osb")
        out_queues = [nc.sync, nc.scalar, nc.sync, nc.scalar, nc.sync, nc.scalar]
        for o in range(OC):
            for idx, (p_hls, p_leaf) in enumerate(pending):
                nc.tensor.matmul(
                    pouts[o],
                    lwg[p_leaf // GL][1][:, p_leaf % GL, o * 128 : (o + 1) * 128],
                    p_hls,
                    start=False,
                    stop=(idx == len(pending) - 1),
                )
            nc.vector.tensor_copy(osb[:, o, :], pouts[o])
            out_queues[o].dma_start(
                out=out_d[o * 128 : (o + 1) * 128, :],
                in_=osb[:, o, :],
            )

    nc.compile()
    return nc


def _to_bf16(a: np.ndarray) -> np.ndarray:
    return np.ascontiguousarray(a, dtype=np.float32).astype(BF16)


def prep_inputs(x, node_w1, node_b1, node_w2, node_b2,
                leaf_w1, leaf_b1, leaf_w2, leaf_b2):
    """Host-side layout prep. Returns (shared weight map, per-core x maps)."""
    x = np.asarray(x, np.float32)
    node_w1 = np.asarray(node_w1, np.float32)
    node_b1 = np.asarray(node_b1, np.float32)
    node_w2 = np.asarray(node_w2, np.float32)
    node_b2 = np.asarray(node_b2, np.float32)
    leaf_w1 = np.asarray(leaf_w1, np.float32)
    leaf_b1 = np.asarray(leaf_b1, np.float32)
    leaf_w2 = np.asarray(leaf_w2, np.float32)
    leaf_b2 = np.asarray(leaf_b2, np.float32)

    # node W1 -> [128, NJ, KC*128]: (p, j, c*128+h') = W1n[c*128+p, j*128+h']
    # (W1n [768, 1008] zero-padded to 1024 columns)
    w1n_flat = node_w1.transpose(1, 0, 2).reshape(IN_DIM, HN)
    w1n_pad = np.zeros((IN_DIM, NJ * 128), np.float32)
    w1n_pad[:, :HN] = w1n_flat
    w1n = w1n_pad.reshape(KC, 128, NJ, 128).transpose(1, 2, 0, 3)
    w1n = w1n.reshape(128, NJ, KC * 128)
    # block-diagonal node W2 [HN, 63], padded to 1024 rows -> [128, NJ, 63]
    w2bd = np.zeros((NJ * 128, N_NODES), np.float32)
    for n in range(N_NODES):
        w2bd[n * NODE_HIDDEN : (n + 1) * NODE_HIDDEN, n] = node_w2[n, :, 0]
    w2bd = w2bd.reshape(NJ, 128, N_NODES).transpose(1, 0, 2)
    # node b1 -> [128, NJ]
    b1n = np.zeros((NJ * 128,), np.float32)
    b1n[:HN] = node_b1.reshape(-1)
    b1n = b1n.reshape(NJ, 128).T
    b2 = node_b2[:, 0]
    b2sp = np.stack([-b2, b2], axis=1)  # [63, 2]

    # leaf W1 grouped GL leaves per DMA: [NG, 128, GL*KC*128] with
    # (g, p, (i, c, h)) = leaf_w1[g*GL+i, c*128+p, h]
    ng = N_LEAVES // GL
    lw1 = leaf_w1.reshape(ng, GL, KC, 128, LEAF_HIDDEN).transpose(0, 3, 1, 2, 4)
    lw1 = lw1.reshape(ng, 128, GL * KC * 128)
    # leaf W2 grouped: [NG, 128, GL*OUT] with (g, p, (i, o)) = leaf_w2[g*GL+i, p, o]
    lw2 = leaf_w2.reshape(ng, GL, LEAF_HIDDEN, OUT_DIM).transpose(0, 2, 1, 3)
    lw2 = lw2.reshape(ng, 128, GL * OUT_DIM)
    b1l = leaf_b1.T  # [128, 64]

    shared = {
        "w1n": _to_bf16(w1n),
        "w2bd": _to_bf16(w2bd),
        "b1n": np.ascontiguousarray(b1n, np.float32),
        "b2sp": np.ascontiguousarray(b2sp, np.float32),
        "mneg": _path_matrix(),
        "lw1": _to_bf16(lw1),
        "b1l": np.ascontiguousarray(b1l, np.float32),
        "lw2": _to_bf16(lw2),
        "b2l": _to_bf16(leaf_b2),
    }
    xts = []
    for c in range(N_CORES):
        xc = x[c * BC : (c + 1) * BC].T  # [768, 512]
        xt = xc.reshape(KC, 128, BC).transpose(1, 0, 2)
        xts.append(_to_bf16(xt))
    return shared, xts


def kernel(**inputs) -> np.ndarray:
    global LAST_RESULT
    shared, xts = prep_inputs(**inputs)
    # leaf_b2 is all-zero for this problem's inputs; the build skips its six
    # rank-64 matmuls then (the first leaf out-chain starts the PSUM banks).
    nc = _build_nc(bool(np.any(np.asarray(inputs["leaf_b2"], np.float32))))
    in_maps = [{**shared, "xt": xts[c]} for c in range(N_CORES)]
    trace = os.environ.get("FFF_TRACE", "0") == "1"
    res = run_bass_kernel_spmd(nc, in_maps, list(range(N_CORES)), trace=trace)
    LAST_RESULT = res
    out = np.empty((BATCH, OUT_DIM), np.float32)
    for c in range(N_CORES):
        out[c * BC : (c + 1) * BC, :] = np.asarray(
            res.results[c]["outT"], dtype=np.float32
        ).T
    return out
